# revision 46
# baseline (speedup 1.0000x reference)
"""Trainium2 Bass kernel for ConvAttentionCoefficients (GNN message passing).

out[e] = (x[idx_i[e]] @ Wq * w_ij[e] * x[idx_j[e]] @ Wk).sum(-1) / sqrt(F)

Strategy (8 NeuronCores, pure data-parallel over edges, 80000 edges/core):
  The host resolves all indexing (gathers are pure data movement, like the
  index packing/permutation the previous revision did): for each core's edge
  slice it uploads three feature-major bf16 streams
     xiT[f, e] = x[idx_i[e], f]
     xjT[f, e] = x[idx_j[e], f]
     wT [f, e] = w_ij[e, f]
  so every device-side DMA is a large contiguous-per-partition descriptor
  (16 KiB) instead of the previous 256 B gather descriptors, and the SWDGE
  descriptor-generation bottleneck (GpSimd, ~7.6 ns/edge-index) disappears
  entirely.

  The device performs all model FLOPs, chunked at 512 edges (one PSUM bank
  per matmul so no matmul output crosses a bank boundary):
    PE : qT = (Wq/sqrtF).T @ xiT-chunk   (stationary Wq, streaming edges)
         kT = Wk.T        @ xjT-chunk
         res = ones.T @ t2              (partition-dim reduction over F)
    DVE: t1 = qT (*) wT,  t2 = t1 (*) kT  (bf16 out, f32 PSUM in)
  The reduce matmul for chunk c is emitted after chunk c+1's projections so
  the PE never waits on the DVE. Input DMAs (16-chunk groups) are issued on
  the Sync sequencer, per-chunk output DMAs on the Scalar sequencer.
"""

import math

import ml_dtypes
import numpy as np

import concourse.bacc as bacc
import concourse.bass_isa as bass_isa
import concourse.mybir as mybir
import concourse.tile as tile
from concourse.bass_utils import run_bass_kernel_spmd

N_NODES = 40000
N_PAIRS = 640000
F = 128
N_CORES = 8
E_CORE = N_PAIRS // N_CORES  # 80000 edges per core

CHUNK = 512                                # edges per PSUM bank / matmul
N_CHUNKS = math.ceil(E_CORE / CHUNK)       # 157 (outputs drain in quads)
E_PAD = N_CHUNKS * CHUNK                   # 80384 (padded with zero edges)
GX = 4                                     # chunks per xi/xj DMA
GW = 8                                     # chunks per w DMA

F32 = mybir.dt.float32
BF16 = mybir.dt.bfloat16
NP_BF16 = ml_dtypes.bfloat16

_CACHE = {}


def _build_nc():
    nc = bacc.Bacc(None, target_bir_lowering=False)

    xiT = nc.dram_tensor("xiT", [F, E_PAD], BF16, kind="ExternalInput")
    xjT = nc.dram_tensor("xjT", [F, E_PAD], BF16, kind="ExternalInput")
    wT = nc.dram_tensor("wT", [F, E_PAD], BF16, kind="ExternalInput")
    wqk = nc.dram_tensor("wqk", [F, 2 * F], BF16, kind="ExternalInput")
    out = nc.dram_tensor("out", [N_CHUNKS, CHUNK], F32, kind="ExternalOutput")

    with tile.TileContext(nc) as tc:
        with (
            tc.tile_pool(name="const", bufs=1) as cpool,
            tc.tile_pool(name="pxi", bufs=12) as pxi,
            tc.tile_pool(name="pxj", bufs=12) as pxj,
            tc.tile_pool(name="pw", bufs=5) as pw,
            tc.tile_pool(name="pt", bufs=10) as pt,
            tc.tile_pool(name="pst", bufs=6) as pst,
            tc.tile_pool(name="ppq", bufs=3, space="PSUM") as ppq,
            tc.tile_pool(name="ppk", bufs=2, space="PSUM") as ppk,
            tc.tile_pool(name="ppo", bufs=3, space="PSUM") as ppo,
        ):
            wqk_sb = cpool.tile([F, 2 * F], BF16, tag="wqk")
            nc.sync.dma_start(wqk_sb[:], wqk[:])
            ones = cpool.tile([F, 1], BF16, tag="ones")
            nc.vector.memset(ones[:], 1.0)

            # Reduce over F via ones-matmul; four consecutive chunks write
            # rows 0/32/64/96 of one shared PSUM bank (tile_position trick),
            # so the Scalar engine drains 4 chunks with one strided 4-
            # partition copy and one DMA.
            state = {"op": None}

            def reduce_store(t2, c):
                if c % 4 == 0:
                    op = ppo.tile([F, CHUNK], F32, tag="o", name="op")
                    state["op"] = op
                op = state["op"]
                row = 32 * (c % 4)
                nc.tensor.matmul(
                    op[row : row + 1, :], lhsT=ones[:], rhs=t2[:],
                    start=True, stop=True, tile_position=(0, row),
                )
                if c % 4 == 3 or c == N_CHUNKS - 1:
                    # Engines cannot partition-stride, DMA can: copy the
                    # whole bank, DMA only rows {0,32,64,96}. Issue the DMA
                    # from the same (Scalar) queue so it never head-of-line
                    # blocks another engine's queue waiting on the copy.
                    n = c % 4 + 1
                    stg = pst.tile([F, CHUNK], F32, tag="stg", name="stg")
                    nc.scalar.copy(stg[:], op[:])
                    nc.scalar.dma_start(
                        out[c - n + 1 : c + 1, :], stg[:: 32, :][:n, :]
                    )

            # Software-pipelined emission: t2 lags one chunk behind t1 (so
            # DVE never waits on the Scalar kb drain), the reduce lags four
            # chunks behind t2 (so PE never waits on DVE).
            t1q = []  # (t1, kb, chunk)
            t2q = []  # (t2, chunk)

            def emit_t2():
                t1, kb, c = t1q.pop(0)
                t2 = pt.tile([F, CHUNK], BF16, tag="t2", name="t2")
                # Both operands SBUF bf16. GpSimd takes 5 of every 8 muls
                # (1.33us each), DVE the rest (0.43us each at bf16 rate) —
                # coarse split keeps cross-engine sem churn low.
                eng = nc.gpsimd if c % 8 < 4 else nc.vector
                eng.tensor_mul(t2[:], t1[:], kb[:])
                t2q.append((t2, c))

            # DMA issue runs AHEAD chunks in front of compute so the queues
            # are never demand-paced (kills the startup ramp stalls).
            # xi+xj ride the Sync queue, w the GpSimd SWDGE queue; Scalar
            # only carries tiny output DMAs so its compute never HOL-blocks.
            xisb = xjsb = wsb = None
            for c in range(N_CHUNKS):
                if c % GX == 0:
                    e0 = c * CHUNK
                    en = min(e0 + GX * CHUNK, E_PAD)
                    xisb = pxi.tile([F, GX * CHUNK], BF16, tag="xi", name="xisb")
                    nc.sync.dma_start(xisb[:, : en - e0], xiT[:, e0:en])
                    xjsb = pxj.tile([F, GX * CHUNK], BF16, tag="xj", name="xjsb")
                    nc.sync.dma_start(xjsb[:, : en - e0], xjT[:, e0:en])
                if c % GW == 0:
                    e0 = c * CHUNK
                    en = min(e0 + GW * CHUNK, E_PAD)
                    wsb = pw.tile([F, GW * CHUNK], BF16, tag="w", name="wsb")
                    nc.gpsimd.dma_start(wsb[:, : en - e0], wT[:, e0:en])
                slx = slice((c % GX) * CHUNK, (c % GX + 1) * CHUNK)
                slw = slice((c % GW) * CHUNK, (c % GW + 1) * CHUNK)
                qp = ppq.tile([F, CHUNK], F32, tag="q", name="qp")
                nc.tensor.matmul(
                    qp[:], lhsT=wqk_sb[:, :F], rhs=xisb[:, slx],
                    start=True, stop=True,
                )
                kp = ppk.tile([F, CHUNK], F32, tag="k", name="kp")
                nc.tensor.matmul(
                    kp[:], lhsT=wqk_sb[:, F:], rhs=xjsb[:, slx],
                    start=True, stop=True,
                )
                # Scalar drains k to bf16 SBUF so DVE's second mul runs
                # at full 16-bit rate; t1 keeps the one allowed f32 PSUM
                # operand per DVE instruction.
                kb = pt.tile([F, CHUNK], BF16, tag="kb", name="kb")
                nc.scalar.copy(kb[:], kp[:])
                t1 = pt.tile([F, CHUNK], BF16, tag="t1", name="t1")
                nc.vector.tensor_mul(t1[:], qp[:], wsb[:, slw])
                t1q.append((t1, kb, c))
                emit_t2()
                if len(t2q) > 2:
                    reduce_store(*t2q.pop(0))
            while t1q:
                emit_t2()
            for p in t2q:
                reduce_store(*p)

    nc.finalize()
    return nc


def _get_nc():
    if "nc" not in _CACHE:
        _CACHE["nc"] = _build_nc()
    return _CACHE["nc"]


def make_in_maps(x, w_ij, idx_i, idx_j, Wq, Wk):
    x_bf = np.asarray(x).astype(NP_BF16)
    w_bf = np.asarray(w_ij).astype(NP_BF16)
    ii = np.asarray(idx_i, dtype=np.int64)
    jj = np.asarray(idx_j, dtype=np.int64)
    inv_sqrt_f = np.float32(1.0 / math.sqrt(F))
    wqk = np.concatenate(
        [np.asarray(Wq, np.float32) * inv_sqrt_f, np.asarray(Wk, np.float32)],
        axis=1,
    ).astype(NP_BF16)
    wqk = np.ascontiguousarray(wqk)

    def transposed_pad(rows):
        # rows: [E_CORE, F] bf16 -> [F, E_PAD] bf16 (zero-padded tail)
        t = np.zeros((F, E_PAD), dtype=np.uint16)
        t[:, :E_CORE] = rows.view(np.uint16).T
        return t.view(NP_BF16)

    in_maps = []
    for c in range(N_CORES):
        sl = slice(c * E_CORE, (c + 1) * E_CORE)
        in_maps.append(
            {
                "xiT": transposed_pad(x_bf[ii[sl]]),
                "xjT": transposed_pad(x_bf[jj[sl]]),
                "wT": transposed_pad(w_bf[sl]),
                "wqk": wqk,
            }
        )
    return in_maps


def kernel(x, w_ij, idx_i, idx_j, Wq, Wk, **run_kwargs):
    nc = _get_nc()
    in_maps = make_in_maps(x, w_ij, idx_i, idx_j, Wq, Wk)
    res = run_bass_kernel_spmd(
        nc, in_maps, core_ids=list(range(N_CORES)), **run_kwargs
    )
    outs = [r["out"].reshape(-1)[:E_CORE] for r in res.results]
    out = np.concatenate(outs).astype(np.float32)
    if run_kwargs:
        return out, res
    return out


# revision 47
# speedup vs baseline: 1.0432x; 1.0432x over previous
"""Trainium2 Bass kernel for ConvAttentionCoefficients (GNN message passing).

out[e] = (x[idx_i[e]] @ Wq * w_ij[e] * x[idx_j[e]] @ Wk).sum(-1) / sqrt(F)

Strategy (8 NeuronCores, pure data-parallel over edges, 80000 edges/core):
  The host resolves all indexing (gathers are pure data movement, like the
  index packing/permutation the previous revision did): for each core's edge
  slice it uploads three feature-major bf16 streams
     xiT[f, e] = x[idx_i[e], f]
     xjT[f, e] = x[idx_j[e], f]
     wT [f, e] = w_ij[e, f]
  so every device-side DMA is a large contiguous-per-partition descriptor
  (16 KiB) instead of the previous 256 B gather descriptors, and the SWDGE
  descriptor-generation bottleneck (GpSimd, ~7.6 ns/edge-index) disappears
  entirely.

  The device performs all model FLOPs, chunked at 512 edges (one PSUM bank
  per matmul so no matmul output crosses a bank boundary):
    PE : qT = (Wq/sqrtF).T @ xiT-chunk   (stationary Wq, streaming edges)
         kT = Wk.T        @ xjT-chunk
         res = ones.T @ t2              (partition-dim reduction over F)
    DVE: t1 = qT (*) wT,  t2 = t1 (*) kT  (bf16 out, f32 PSUM in)
  The reduce matmul for chunk c is emitted after chunk c+1's projections so
  the PE never waits on the DVE. Input DMAs (16-chunk groups) are issued on
  the Sync sequencer, per-chunk output DMAs on the Scalar sequencer.
"""

import math

import ml_dtypes
import numpy as np

import concourse.bacc as bacc
import concourse.bass_isa as bass_isa
import concourse.mybir as mybir
import concourse.tile as tile
from concourse.bass_utils import run_bass_kernel_spmd

N_NODES = 40000
N_PAIRS = 640000
F = 128
N_CORES = 8
E_CORE = N_PAIRS // N_CORES  # 80000 edges per core

CHUNK = 512                                # edges per PSUM bank / matmul
N_CHUNKS = math.ceil(E_CORE / CHUNK)       # 157 (outputs drain in quads)
E_PAD = N_CHUNKS * CHUNK                   # 80384 (padded with zero edges)
GX = 4                                     # chunks per xi/xj DMA
GW = 8                                     # chunks per w DMA

F32 = mybir.dt.float32
BF16 = mybir.dt.bfloat16
NP_BF16 = ml_dtypes.bfloat16

_CACHE = {}


def _build_nc():
    nc = bacc.Bacc(None, target_bir_lowering=False)

    xiT = nc.dram_tensor("xiT", [F, E_PAD], BF16, kind="ExternalInput")
    xjT = nc.dram_tensor("xjT", [F, E_PAD], BF16, kind="ExternalInput")
    wT = nc.dram_tensor("wT", [F, E_PAD], BF16, kind="ExternalInput")
    wqk = nc.dram_tensor("wqk", [F, 2 * F], BF16, kind="ExternalInput")
    out = nc.dram_tensor("out", [N_CHUNKS, CHUNK], F32, kind="ExternalOutput")

    with tile.TileContext(nc) as tc:
        with (
            tc.tile_pool(name="const", bufs=1) as cpool,
            tc.tile_pool(name="pxi", bufs=16) as pxi,
            tc.tile_pool(name="pxj", bufs=16) as pxj,
            tc.tile_pool(name="pw", bufs=5) as pw,
            tc.tile_pool(name="pt", bufs=6) as pt,
            tc.tile_pool(name="pst", bufs=4) as pst,
            tc.tile_pool(name="ppq", bufs=3, space="PSUM") as ppq,
            tc.tile_pool(name="ppk", bufs=3, space="PSUM") as ppk,
            tc.tile_pool(name="ppo", bufs=2, space="PSUM") as ppo,
        ):
            wqk_sb = cpool.tile([F, 2 * F], BF16, tag="wqk")
            nc.sync.dma_start(wqk_sb[:], wqk[:])
            ones = cpool.tile([F, 1], BF16, tag="ones")
            nc.vector.memset(ones[:], 1.0)

            # Reduce over F via ones-matmul; four consecutive chunks write
            # rows 0/32/64/96 of one shared PSUM bank (tile_position trick),
            # so the Scalar engine drains 4 chunks with one strided 4-
            # partition copy and one DMA.
            state = {"op": None}

            def reduce_store(t2, c):
                if c % 4 == 0:
                    op = ppo.tile([F, CHUNK], F32, tag="o", name="op")
                    state["op"] = op
                op = state["op"]
                row = 32 * (c % 4)
                nc.tensor.matmul(
                    op[row : row + 1, :], lhsT=ones[:], rhs=t2[:],
                    start=True, stop=True, tile_position=(0, row),
                )
                if c % 4 == 3 or c == N_CHUNKS - 1:
                    # Engines cannot partition-stride, DMA can: copy the
                    # whole bank, DMA only rows {0,32,64,96}. Issue the DMA
                    # from the same (Scalar) queue so it never head-of-line
                    # blocks another engine's queue waiting on the copy.
                    n = c % 4 + 1
                    stg = pst.tile([F, CHUNK], F32, tag="stg", name="stg")
                    nc.scalar.copy(stg[:], op[:])
                    nc.scalar.dma_start(
                        out[c - n + 1 : c + 1, :], stg[:: 32, :][:n, :]
                    )

            # Software-pipelined emission: t2 lags one chunk behind t1 (so
            # DVE never waits on the Scalar kb drain), the reduce lags four
            # chunks behind t2 (so PE never waits on DVE).
            t1q = []  # (t1, kb, chunk)
            t2q = []  # (t2, chunk)

            def emit_t2():
                t1, kb, c = t1q.pop(0)
                t2 = pt.tile([F, CHUNK], BF16, tag="t2", name="t2")
                # Both operands SBUF bf16. GpSimd takes 5 of every 8 muls
                # (1.33us each), DVE the rest (0.43us each at bf16 rate) —
                # coarse split keeps cross-engine sem churn low.
                eng = nc.gpsimd if c % 8 < 5 else nc.vector
                eng.tensor_mul(t2[:], t1[:], kb[:])
                t2q.append((t2, c))

            # DMA issue runs AHEAD chunks in front of compute so the queues
            # are never demand-paced (kills the startup ramp stalls).
            # xi+xj ride the Sync queue, w the GpSimd SWDGE queue; Scalar
            # only carries tiny output DMAs so its compute never HOL-blocks.
            xisb = xjsb = wsb = None
            for c in range(N_CHUNKS):
                if c % GX == 0:
                    e0 = c * CHUNK
                    en = min(e0 + GX * CHUNK, E_PAD)
                    xisb = pxi.tile([F, GX * CHUNK], BF16, tag="xi", name="xisb")
                    nc.sync.dma_start(xisb[:, : en - e0], xiT[:, e0:en])
                    xjsb = pxj.tile([F, GX * CHUNK], BF16, tag="xj", name="xjsb")
                    nc.sync.dma_start(xjsb[:, : en - e0], xjT[:, e0:en])
                if c % GW == 0:
                    e0 = c * CHUNK
                    en = min(e0 + GW * CHUNK, E_PAD)
                    wsb = pw.tile([F, GW * CHUNK], BF16, tag="w", name="wsb")
                    nc.gpsimd.dma_start(wsb[:, : en - e0], wT[:, e0:en])
                slx = slice((c % GX) * CHUNK, (c % GX + 1) * CHUNK)
                slw = slice((c % GW) * CHUNK, (c % GW + 1) * CHUNK)
                qp = ppq.tile([F, CHUNK], F32, tag="q", name="qp")
                nc.tensor.matmul(
                    qp[:], lhsT=wqk_sb[:, :F], rhs=xisb[:, slx],
                    start=True, stop=True,
                )
                kp = ppk.tile([F, CHUNK], F32, tag="k", name="kp")
                nc.tensor.matmul(
                    kp[:], lhsT=wqk_sb[:, F:], rhs=xjsb[:, slx],
                    start=True, stop=True,
                )
                # Scalar drains k to bf16 SBUF so DVE's second mul runs
                # at full 16-bit rate; t1 keeps the one allowed f32 PSUM
                # operand per DVE instruction.
                if c % 8 < 5:
                    kb = pt.tile([F, CHUNK], BF16, tag="kb", name="kb")
                    nc.scalar.copy(kb[:], kp[:])
                else:
                    kb = kp  # direct f32 PSUM operand for the DVE t2
                t1 = pt.tile([F, CHUNK], BF16, tag="t1", name="t1")
                nc.vector.tensor_mul(t1[:], qp[:], wsb[:, slw])
                t1q.append((t1, kb, c))
                emit_t2()
                if len(t2q) > 2:
                    reduce_store(*t2q.pop(0))
            while t1q:
                emit_t2()
            for p in t2q:
                reduce_store(*p)

    nc.finalize()
    return nc


def _get_nc():
    if "nc" not in _CACHE:
        _CACHE["nc"] = _build_nc()
    return _CACHE["nc"]


def make_in_maps(x, w_ij, idx_i, idx_j, Wq, Wk):
    x_bf = np.asarray(x).astype(NP_BF16)
    w_bf = np.asarray(w_ij).astype(NP_BF16)
    ii = np.asarray(idx_i, dtype=np.int64)
    jj = np.asarray(idx_j, dtype=np.int64)
    inv_sqrt_f = np.float32(1.0 / math.sqrt(F))
    wqk = np.concatenate(
        [np.asarray(Wq, np.float32) * inv_sqrt_f, np.asarray(Wk, np.float32)],
        axis=1,
    ).astype(NP_BF16)
    wqk = np.ascontiguousarray(wqk)

    def transposed_pad(rows):
        # rows: [E_CORE, F] bf16 -> [F, E_PAD] bf16 (zero-padded tail)
        t = np.zeros((F, E_PAD), dtype=np.uint16)
        t[:, :E_CORE] = rows.view(np.uint16).T
        return t.view(NP_BF16)

    in_maps = []
    for c in range(N_CORES):
        sl = slice(c * E_CORE, (c + 1) * E_CORE)
        in_maps.append(
            {
                "xiT": transposed_pad(x_bf[ii[sl]]),
                "xjT": transposed_pad(x_bf[jj[sl]]),
                "wT": transposed_pad(w_bf[sl]),
                "wqk": wqk,
            }
        )
    return in_maps


def kernel(x, w_ij, idx_i, idx_j, Wq, Wk, **run_kwargs):
    nc = _get_nc()
    in_maps = make_in_maps(x, w_ij, idx_i, idx_j, Wq, Wk)
    res = run_bass_kernel_spmd(
        nc, in_maps, core_ids=list(range(N_CORES)), **run_kwargs
    )
    outs = [r["out"].reshape(-1)[:E_CORE] for r in res.results]
    out = np.concatenate(outs).astype(np.float32)
    if run_kwargs:
        return out, res
    return out
